# revision 58
# baseline (speedup 1.0000x reference)
"""Chamfer distance (B=4, N1=N2=8192, D=3) on 8 NeuronCores.

Sharding: core = b*2 + h handles xyz1[b, h*4096:(h+1)*4096] vs all of xyz2[b].

Per-core device kernel:
  - Host lifts points to K=24 bf16 vectors (3-way hi/mid/lo split per fp32
    factor) so a single bf16 matmul produces NEGATED squared distances in
    PSUM: -d[i,j] = -|x_i|^2 - |y_j|^2 + (2x_i).y_j, accurate to ~2^-27.
  - K=24 <= 32, so the PE runs in 32x128 row-tiling mode: 4 concurrent
    matmuls (tile_position (32g, 0)) fill a 4-bank PSUM group [128, 2048]
    in about one matmul's time. The lifted operands are replicated at SBUF
    partition offsets 0/32/64/96 to feed the four row-groups.
  - With negated distances every min becomes a max:
      dist1[i]: elementwise TT-max over j-groups into rowacc[128, 2048],
                folded + tensor_reduce(max) per 128-row block.
      dist2[j]: elementwise TT-max over i-blocks into colacc[gc], folded by
                gpsimd partition_all_reduce(max) at the end.
  - PSUM egress: ACT copies each group to fp16 SBUF (ScalarE is the only
    max-capable-adjacent engine with spare cycles; GPSIMD TensorTensor and
    DMA accum max are both rejected by this walrus), then DVE runs both
    reduction passes as 2x-mode fp16 tensor_tensor(max) -- the DVE is the
    binding engine at ~92% occupancy.
"""

import os
import numpy as np

B, N1, N2, D = 4, 8192, 8192, 3
N_CORES = 8
I_PER_CORE = N1 // 2          # 4096 xyz1 rows per core
J = N2                        # 8192 xyz2 points (full)
IB = I_PER_CORE // 128        # 32 i-blocks
GW = 2048                     # PSUM group width (4 banks, 4 packed matmuls)
NG = J // GW                  # 4 column groups per i-block
KDIM = 24                     # bf16 3-way-split lifted contraction depth
NEG_INF_F16 = -60000.0

# Row accumulation: 'V' = fp16 2x tensor_tensor + explicit fold (best);
# 'M' = per-group vector.max top-8 (measured 1x rate -> slower);
# 'T' = tensor_tensor_reduce (compiles but crashes TRN2 at runtime).
ROW_MODE = os.environ.get("CHAMFER_ROW", "V")

_CACHE = {}


def _build_program():
    from contextlib import ExitStack

    import concourse.bacc as bacc
    import concourse.tile as tile
    from concourse import mybir
    from concourse import bass_isa

    f32 = mybir.dt.float32
    f16 = mybir.dt.float16
    bf16 = mybir.dt.bfloat16
    MAX = mybir.AluOpType.max

    nc = bacc.Bacc("TRN2", num_swdge_queues=2)
    # Lifted operands for all four PE row-groups: partitions 32g+k (k<24)
    # hold lifted row k. Split into two tensors so the two DMAs overlap.
    l1_d = nc.declare_dram_parameter("lifted1", [128, I_PER_CORE], bf16, isOutput=False)
    l2_d = nc.declare_dram_parameter("lifted2", [128, J], bf16, isOutput=False)
    d1_d = nc.declare_dram_parameter("d1out", [128, IB], f32, isOutput=True)
    d2_d = nc.declare_dram_parameter("d2out", [1, J], f16, isOutput=True)

    with tile.TileContext(nc) as tc, ExitStack() as ctx:
        const = ctx.enter_context(tc.tile_pool(name="const", bufs=1))
        psum = ctx.enter_context(tc.tile_pool(name="psum", bufs=2, space="PSUM"))
        cpool = ctx.enter_context(tc.tile_pool(name="copies", bufs=6))
        rpool = ctx.enter_context(tc.tile_pool(name="rowacc", bufs=3))
        fpool = ctx.enter_context(tc.tile_pool(name="fold", bufs=2))

        l1sb = const.tile([128, I_PER_CORE], bf16, tag="lifted1")
        l2sb = const.tile([128, J], bf16, tag="lifted2")
        # chunked and interleaved so the first matmuls' slices land first;
        # tiny leading chunks let the very first matmul start early
        l1cuts = [0, 128, 1024, 2048, 3072, I_PER_CORE]
        l2cuts = [0, 512, 2048, 4096, 6144, J]
        for c in range(5):
            nc.sync.dma_start(
                l1sb[:, l1cuts[c]:l1cuts[c + 1]], l1_d[:, l1cuts[c]:l1cuts[c + 1]]
            )
            nc.sync.dma_start(
                l2sb[:, l2cuts[c]:l2cuts[c + 1]], l2_d[:, l2cuts[c]:l2cuts[c + 1]]
            )

        d1sb = const.tile([128, IB], f32, tag="d1sb")

        # colacc needs no memset: the ib=0 ACT copies write it directly
        colacc = []
        for gc in range(NG):
            t = const.tile([128, GW], f16, tag=f"colacc{gc}")
            colacc.append(t)

        for ib in range(IB):
            if ROW_MODE == "M":
                rt = rpool.tile([128, NG * 8], f16, tag="rowtop")
            else:
                rowacc = rpool.tile([128, GW], f16, tag="rowacc")
                if ib % 4 == 0:
                    # folded 512-wide row maxes for 4 blocks, reduced at once
                    qb = rpool.tile([128, 4 * 512], f16, tag="quadred")
            last_cps = []
            for gc in range(NG):
                pt = psum.tile([128, GW], f32, tag="pt")
                for g in range(4):
                    jlo = gc * GW + g * 512
                    nc.tensor.matmul(
                        pt[:, g * 512:(g + 1) * 512],
                        l1sb[32 * g:32 * g + KDIM, ib * 128:(ib + 1) * 128],
                        l2sb[32 * g:32 * g + KDIM, jlo:jlo + 512],
                        start=True,
                        stop=True,
                        tile_position=(32 * g, 0),
                    )
                if ib == 0:
                    cp = colacc[gc]  # ib=0 copies initialize colacc directly
                elif ROW_MODE != "M" and gc == 0:
                    cp = rowacc      # ACT copy doubles as rowacc init
                else:
                    cp = cpool.tile([128, GW], f16, tag="cp")
                nc.scalar.copy(cp[:], pt[:])
                if ROW_MODE == "M":
                    if ib != 0:
                        nc.vector.tensor_tensor(
                            colacc[gc][:], colacc[gc][:], cp[:], op=MAX
                        )
                    nc.vector.max(rt[:, gc * 8:(gc + 1) * 8], cp[:])
                    continue
                if ib == 0:
                    # rowacc built from the colacc inits; no col TT needed.
                    # gc=0 uses a 4x-mode copy so DVE starts after ONE ACT
                    # copy instead of two.
                    if gc == 0:
                        nc.vector.tensor_copy(rowacc[:], colacc[0][:])
                    else:
                        nc.vector.tensor_tensor(
                            rowacc[:], rowacc[:], colacc[gc][:], op=MAX
                        )
                    continue
                if gc != 0 and ib != IB - 1:
                    nc.vector.tensor_tensor(rowacc[:], rowacc[:], cp[:], op=MAX)
                nc.vector.tensor_tensor(colacc[gc][:], colacc[gc][:], cp[:], op=MAX)
                if ib == IB - 1:
                    last_cps.append(cp)
            if ROW_MODE == "M":
                nc.vector.tensor_reduce(
                    d1sb[:, ib:ib + 1], rt[:], axis=mybir.AxisListType.X, op=MAX
                )
                continue
            if ib == IB - 1:
                # last block: col TTs were issued first so the gpsimd
                # partition folds can start; do the deferred row TTs now
                for cp in last_cps[1:]:
                    nc.vector.tensor_tensor(rowacc[:], rowacc[:], cp[:], op=MAX)
            # fold rowacc [128, GW] -> 512-wide slot in qb; one reduce per 4
            nc.vector.tensor_tensor(
                rowacc[:, 0:1024], rowacc[:, 0:1024], rowacc[:, 1024:2048], op=MAX
            )
            nc.vector.tensor_tensor(
                qb[:, (ib % 4) * 512:(ib % 4 + 1) * 512],
                rowacc[:, 0:512], rowacc[:, 512:1024], op=MAX,
            )
            if ib % 4 == 3:
                nc.vector.tensor_reduce(
                    d1sb[:, ib - 3:ib + 1],
                    qb[:].rearrange("p (i w) -> p i w", i=4),
                    axis=mybir.AxisListType.X, op=MAX,
                )

        nc.sync.dma_start(d1_d[:], d1sb[:])

        for gc in range(NG):
            fold = fpool.tile([128, GW], f16, tag="fold")
            nc.gpsimd.partition_all_reduce(
                fold[:], colacc[gc][:], 128, bass_isa.ReduceOp.max
            )
            nc.sync.dma_start(d2_d[0:1, gc * GW:(gc + 1) * GW], fold[0:1, :])

    nc.compile()
    return nc


def _get_program():
    if "nc" not in _CACHE:
        _CACHE["nc"] = _build_program()
    return _CACHE["nc"]


def _bf16_split3(v):
    import ml_dtypes

    bf16 = ml_dtypes.bfloat16
    hi = v.astype(bf16).astype(np.float32)
    r = v - hi
    mid = r.astype(bf16).astype(np.float32)
    lo = (r - mid).astype(bf16).astype(np.float32)
    return hi, mid, lo


def _lift(xyz1_half, xyz2_full):
    """Pack [lifted1 | lifted2] into one [128, n1+n2] bf16 array, the 24
    lifted rows replicated at partition offsets 0/32/64/96 for the four PE
    row-groups.

    -d[i,j] = -sq1_i - sq2_j + (2*x_i).y_j, every fp32 factor split 3-way
    into bf16 (hi, mid, lo); product pairs keep all terms down to ~2^-27:
    hh, hm, mh, hl, lh, mm per coordinate.
    """
    import ml_dtypes

    x1 = np.ascontiguousarray(xyz1_half, dtype=np.float32)
    x2 = np.ascontiguousarray(xyz2_full, dtype=np.float32)
    sq1 = (x1 * x1).sum(-1)
    sq2 = (x2 * x2).sum(-1)
    n1 = x1.shape[0]
    n2 = x2.shape[0]
    A = np.empty((KDIM, n1), np.float32)
    B_ = np.empty((KDIM, n2), np.float32)
    A[0], A[1], A[2] = _bf16_split3(-sq1)
    B_[0:3] = 1.0
    A[3:6] = 1.0
    B_[3], B_[4], B_[5] = _bf16_split3(-sq2)
    for d in range(3):
        ah, am, al = _bf16_split3(2.0 * x1[:, d])
        bh, bm, bl = _bf16_split3(x2[:, d])
        r = 6 + 6 * d
        A[r + 0], B_[r + 0] = ah, bh
        A[r + 1], B_[r + 1] = ah, bm
        A[r + 2], B_[r + 2] = am, bh
        A[r + 3], B_[r + 3] = ah, bl
        A[r + 4], B_[r + 4] = al, bh
        A[r + 5], B_[r + 5] = am, bm
    lifted1 = np.zeros((128, n1), ml_dtypes.bfloat16)
    lifted2 = np.zeros((128, n2), ml_dtypes.bfloat16)
    for g in range(4):
        lifted1[32 * g:32 * g + KDIM] = A
        lifted2[32 * g:32 * g + KDIM] = B_
    return lifted1, lifted2


def kernel(xyz1, xyz2):
    from concourse.bass_utils import run_bass_kernel_spmd

    xyz1 = np.asarray(xyz1, dtype=np.float32)
    xyz2 = np.asarray(xyz2, dtype=np.float32)

    nc = _get_program()
    in_maps = []
    for core in range(N_CORES):
        b, h = divmod(core, 2)
        l1, l2 = _lift(xyz1[b, h * I_PER_CORE:(h + 1) * I_PER_CORE], xyz2[b])
        in_maps.append({"lifted1": l1, "lifted2": l2})

    trace = bool(int(os.environ.get("CHAMFER_TRACE", "0")))
    out = run_bass_kernel_spmd(nc, in_maps, list(range(N_CORES)), trace=trace)
    _CACHE["last_exec_ns"] = out.exec_time_ns
    _CACHE["last_results"] = out
    res = out.results

    d1_sum = 0.0
    d2_sum = 0.0
    for b in range(B):
        for h in range(2):
            m1 = res[b * 2 + h]["d1out"]  # [128, IB], max_j of -d
            d1_sum += -m1.astype(np.float64).sum()
        m2a = res[b * 2 + 0]["d2out"][0].astype(np.float32)  # [J], max over half i
        m2b = res[b * 2 + 1]["d2out"][0].astype(np.float32)
        d2_sum += -np.maximum(m2a, m2b).astype(np.float64).sum()

    mean1 = d1_sum / (B * N1)
    mean2 = d2_sum / (B * N2)
    return np.float32(mean1 + mean2)


# revision 59
# speedup vs baseline: 1.0042x; 1.0042x over previous
"""Chamfer distance (B=4, N1=N2=8192, D=3) on 8 NeuronCores.

Sharding: core = b*2 + h handles xyz1[b, h*4096:(h+1)*4096] vs all of xyz2[b].

Per-core device kernel:
  - Host lifts points to K=24 bf16 vectors (3-way hi/mid/lo split per fp32
    factor) so a single bf16 matmul produces NEGATED squared distances in
    PSUM: -d[i,j] = -|x_i|^2 - |y_j|^2 + (2x_i).y_j, accurate to ~2^-27.
  - K=24 <= 32, so the PE runs in 32x128 row-tiling mode: 4 concurrent
    matmuls (tile_position (32g, 0)) fill a 4-bank PSUM group [128, 2048]
    in about one matmul's time. The lifted operands are replicated at SBUF
    partition offsets 0/32/64/96 to feed the four row-groups.
  - With negated distances every min becomes a max:
      dist1[i]: elementwise TT-max over j-groups into rowacc[128, 2048],
                folded + tensor_reduce(max) per 128-row block.
      dist2[j]: elementwise TT-max over i-blocks into colacc[gc], folded by
                gpsimd partition_all_reduce(max) at the end.
  - PSUM egress: ACT copies each group to fp16 SBUF (ScalarE is the only
    max-capable-adjacent engine with spare cycles; GPSIMD TensorTensor and
    DMA accum max are both rejected by this walrus), then DVE runs both
    reduction passes as 2x-mode fp16 tensor_tensor(max) -- the DVE is the
    binding engine at ~92% occupancy.
"""

import os
import numpy as np

B, N1, N2, D = 4, 8192, 8192, 3
N_CORES = 8
I_PER_CORE = N1 // 2          # 4096 xyz1 rows per core
J = N2                        # 8192 xyz2 points (full)
IB = I_PER_CORE // 128        # 32 i-blocks
GW = 2048                     # PSUM group width (4 banks, 4 packed matmuls)
NG = J // GW                  # 4 column groups per i-block
KDIM = 24                     # bf16 3-way-split lifted contraction depth
NEG_INF_F16 = -60000.0

# Row accumulation: 'V' = fp16 2x tensor_tensor + explicit fold (best);
# 'M' = per-group vector.max top-8 (measured 1x rate -> slower);
# 'T' = tensor_tensor_reduce (compiles but crashes TRN2 at runtime).
ROW_MODE = os.environ.get("CHAMFER_ROW", "V")

_CACHE = {}


def _build_program():
    from contextlib import ExitStack

    import concourse.bacc as bacc
    import concourse.tile as tile
    from concourse import mybir
    from concourse import bass_isa

    f32 = mybir.dt.float32
    f16 = mybir.dt.float16
    bf16 = mybir.dt.bfloat16
    MAX = mybir.AluOpType.max

    nc = bacc.Bacc("TRN2", num_swdge_queues=2)
    # Lifted operands for all four PE row-groups: partitions 32g+k (k<24)
    # hold lifted row k. Split into two tensors so the two DMAs overlap.
    l1_d = nc.declare_dram_parameter("lifted1", [128, I_PER_CORE], bf16, isOutput=False)
    l2_d = nc.declare_dram_parameter("lifted2", [128, J], bf16, isOutput=False)
    d1_d = nc.declare_dram_parameter("d1out", [128, IB], f32, isOutput=True)
    d2_d = nc.declare_dram_parameter("d2out", [1, J], f16, isOutput=True)

    with tile.TileContext(nc) as tc, ExitStack() as ctx:
        const = ctx.enter_context(tc.tile_pool(name="const", bufs=1))
        psum = ctx.enter_context(tc.tile_pool(name="psum", bufs=2, space="PSUM"))
        cpool = ctx.enter_context(tc.tile_pool(name="copies", bufs=6))
        rpool = ctx.enter_context(tc.tile_pool(name="rowacc", bufs=3))
        fpool = ctx.enter_context(tc.tile_pool(name="fold", bufs=2))

        l1sb = const.tile([128, I_PER_CORE], bf16, tag="lifted1")
        l2sb = const.tile([128, J], bf16, tag="lifted2")
        # chunked and interleaved so the first matmuls' slices land first;
        # tiny leading chunks let the very first matmul start early
        l1cuts = [0, 128, 1024, 2048, 3072, I_PER_CORE]
        l2cuts = [0, 512, 2048, 4096, 6144, J]
        for c in range(5):
            nc.sync.dma_start(
                l1sb[:, l1cuts[c]:l1cuts[c + 1]], l1_d[:, l1cuts[c]:l1cuts[c + 1]]
            )
            nc.sync.dma_start(
                l2sb[:, l2cuts[c]:l2cuts[c + 1]], l2_d[:, l2cuts[c]:l2cuts[c + 1]]
            )

        d1sb = const.tile([128, IB], f32, tag="d1sb")

        # colacc needs no memset: the ib=0 ACT copies write it directly
        colacc = []
        for gc in range(NG):
            t = const.tile([128, GW], f16, tag=f"colacc{gc}")
            colacc.append(t)

        for ib in range(IB):
            if ROW_MODE == "M":
                rt = rpool.tile([128, NG * 8], f16, tag="rowtop")
            else:
                rowacc = rpool.tile([128, GW], f16, tag="rowacc")
            last_cps = []
            for gc in range(NG):
                pt = psum.tile([128, GW], f32, tag="pt")
                for g in range(4):
                    jlo = gc * GW + g * 512
                    nc.tensor.matmul(
                        pt[:, g * 512:(g + 1) * 512],
                        l1sb[32 * g:32 * g + KDIM, ib * 128:(ib + 1) * 128],
                        l2sb[32 * g:32 * g + KDIM, jlo:jlo + 512],
                        start=True,
                        stop=True,
                        tile_position=(32 * g, 0),
                    )
                if ib == 0:
                    cp = colacc[gc]  # ib=0 copies initialize colacc directly
                elif ROW_MODE != "M" and gc == 0:
                    cp = rowacc      # ACT copy doubles as rowacc init
                else:
                    cp = cpool.tile([128, GW], f16, tag="cp")
                nc.scalar.copy(cp[:], pt[:])
                if ROW_MODE == "M":
                    if ib != 0:
                        nc.vector.tensor_tensor(
                            colacc[gc][:], colacc[gc][:], cp[:], op=MAX
                        )
                    nc.vector.max(rt[:, gc * 8:(gc + 1) * 8], cp[:])
                    continue
                if ib == 0:
                    # rowacc built from the colacc inits; no col TT needed.
                    # gc=0 uses a 4x-mode copy so DVE starts after ONE ACT
                    # copy instead of two.
                    if gc == 0:
                        nc.vector.tensor_copy(rowacc[:], colacc[0][:])
                    else:
                        nc.vector.tensor_tensor(
                            rowacc[:], rowacc[:], colacc[gc][:], op=MAX
                        )
                    continue
                if gc != 0 and ib != IB - 1:
                    nc.vector.tensor_tensor(rowacc[:], rowacc[:], cp[:], op=MAX)
                nc.vector.tensor_tensor(colacc[gc][:], colacc[gc][:], cp[:], op=MAX)
                if ib == IB - 1:
                    last_cps.append(cp)
            if ROW_MODE == "M":
                nc.vector.tensor_reduce(
                    d1sb[:, ib:ib + 1], rt[:], axis=mybir.AxisListType.X, op=MAX
                )
                continue
            if ib == IB - 1:
                # last block: col TTs were issued first so the gpsimd
                # partition folds can start; do the deferred row TTs now
                for cp in last_cps[1:]:
                    nc.vector.tensor_tensor(rowacc[:], rowacc[:], cp[:], op=MAX)
            # fold rowacc [128, GW] -> d1sb[:, ib]
            w = GW
            while w > 512:
                w //= 2
                nc.vector.tensor_tensor(
                    rowacc[:, 0:w], rowacc[:, 0:w], rowacc[:, w:2 * w], op=MAX
                )
            nc.vector.tensor_reduce(
                d1sb[:, ib:ib + 1], rowacc[:, 0:w],
                axis=mybir.AxisListType.X, op=MAX,
            )

        nc.sync.dma_start(d1_d[:], d1sb[:])

        for gc in range(NG):
            fold = fpool.tile([128, GW], f16, tag="fold")
            nc.gpsimd.partition_all_reduce(
                fold[:], colacc[gc][:], 128, bass_isa.ReduceOp.max
            )
            nc.sync.dma_start(d2_d[0:1, gc * GW:(gc + 1) * GW], fold[0:1, :])

    nc.compile()
    return nc


def _get_program():
    if "nc" not in _CACHE:
        _CACHE["nc"] = _build_program()
    return _CACHE["nc"]


def _bf16_split3(v):
    import ml_dtypes

    bf16 = ml_dtypes.bfloat16
    hi = v.astype(bf16).astype(np.float32)
    r = v - hi
    mid = r.astype(bf16).astype(np.float32)
    lo = (r - mid).astype(bf16).astype(np.float32)
    return hi, mid, lo


def _lift(xyz1_half, xyz2_full):
    """Pack [lifted1 | lifted2] into one [128, n1+n2] bf16 array, the 24
    lifted rows replicated at partition offsets 0/32/64/96 for the four PE
    row-groups.

    -d[i,j] = -sq1_i - sq2_j + (2*x_i).y_j, every fp32 factor split 3-way
    into bf16 (hi, mid, lo); product pairs keep all terms down to ~2^-27:
    hh, hm, mh, hl, lh, mm per coordinate.
    """
    import ml_dtypes

    x1 = np.ascontiguousarray(xyz1_half, dtype=np.float32)
    x2 = np.ascontiguousarray(xyz2_full, dtype=np.float32)
    sq1 = (x1 * x1).sum(-1)
    sq2 = (x2 * x2).sum(-1)
    n1 = x1.shape[0]
    n2 = x2.shape[0]
    A = np.empty((KDIM, n1), np.float32)
    B_ = np.empty((KDIM, n2), np.float32)
    A[0], A[1], A[2] = _bf16_split3(-sq1)
    B_[0:3] = 1.0
    A[3:6] = 1.0
    B_[3], B_[4], B_[5] = _bf16_split3(-sq2)
    for d in range(3):
        ah, am, al = _bf16_split3(2.0 * x1[:, d])
        bh, bm, bl = _bf16_split3(x2[:, d])
        r = 6 + 6 * d
        A[r + 0], B_[r + 0] = ah, bh
        A[r + 1], B_[r + 1] = ah, bm
        A[r + 2], B_[r + 2] = am, bh
        A[r + 3], B_[r + 3] = ah, bl
        A[r + 4], B_[r + 4] = al, bh
        A[r + 5], B_[r + 5] = am, bm
    lifted1 = np.zeros((128, n1), ml_dtypes.bfloat16)
    lifted2 = np.zeros((128, n2), ml_dtypes.bfloat16)
    for g in range(4):
        lifted1[32 * g:32 * g + KDIM] = A
        lifted2[32 * g:32 * g + KDIM] = B_
    return lifted1, lifted2


def kernel(xyz1, xyz2):
    from concourse.bass_utils import run_bass_kernel_spmd

    xyz1 = np.asarray(xyz1, dtype=np.float32)
    xyz2 = np.asarray(xyz2, dtype=np.float32)

    nc = _get_program()
    in_maps = []
    for core in range(N_CORES):
        b, h = divmod(core, 2)
        l1, l2 = _lift(xyz1[b, h * I_PER_CORE:(h + 1) * I_PER_CORE], xyz2[b])
        in_maps.append({"lifted1": l1, "lifted2": l2})

    trace = bool(int(os.environ.get("CHAMFER_TRACE", "0")))
    out = run_bass_kernel_spmd(nc, in_maps, list(range(N_CORES)), trace=trace)
    _CACHE["last_exec_ns"] = out.exec_time_ns
    _CACHE["last_results"] = out
    res = out.results

    d1_sum = 0.0
    d2_sum = 0.0
    for b in range(B):
        for h in range(2):
            m1 = res[b * 2 + h]["d1out"]  # [128, IB], max_j of -d
            d1_sum += -m1.astype(np.float64).sum()
        m2a = res[b * 2 + 0]["d2out"][0].astype(np.float32)  # [J], max over half i
        m2b = res[b * 2 + 1]["d2out"][0].astype(np.float32)
        d2_sum += -np.maximum(m2a, m2b).astype(np.float64).sum()

    mean1 = d1_sum / (B * N1)
    mean2 = d2_sum / (B * N2)
    return np.float32(mean1 + mean2)
